# revision 135
# baseline (speedup 1.0000x reference)
"""Trainium2 Bass kernel for nn_AttentionPool (topk_masking), t-major design.

Full computation:
    xn     = mean_V(x).T                    (N, T, C)
    qk     = xn @ W + b ; split into q, k   per-head
    att    = q @ k^T / sqrt(hd)
    scores = mean(att, heads+keys)          (N, T)
    idx,v  = top_k(scores, 128)  (desc, stable)
    out    = gather(x, idx, axis=T) * sigmoid(v)

Key algebraic collapse (same as the c-major predecessor): scores is a mean
over heads AND keys, so the TxT attention never materializes:
    scores[t] = alpha * (xnS[:, t] . u) + beta
with u = Wq ksum, ksum = Wk^T xsum / V + T*bk, beta = scale_s * (bq . ksum).

Sharding: data-parallel over batch N=32 across 8 cores (4 samples each);
W/b replicated; no cross-core communication.

v2 layout (t-major): x is uploaded as (B, T, C, V) f32 (host transpose).
Loads stream (128t, 32c, 25v) chunks; DVE reduces v -> xnT (t, c) tiles
(bit-identical per-(c,t) sums to the c-major kernel), and the chunk is
converted to fp16 resident (128t, 256c, 25v) payload tiles (ACT mostly,
3/8 on the otherwise-idle Pool; the last sample's final two tiles go
entirely to Pool so its score chain finds a clear ACT queue). xnT tiles
are PE-transposed into xn_c (c, T) so the score chain (xsum via PE
ones-matmuls on xnT, ksum/u/beta via PE with compacted Wq/Wk, score
broadcast matmul, rank/one-hot split across DVE k0,k2 / ACT k1,k3) keeps
the validated c-major chain's arithmetic: top-k indices match the oracle
exactly (verified bit-level via the --debug path). The real TRN2 Pool
engine supports neither TensorScalarPtr nor free-axis reduce nor PSUM
access, so it can only take tensor_copy offload.

The gather is the point of the layout change: the rank pass's one-hot
tiles pk[t, j] already have t on partitions, so gather(x, idx) is plain
PE matmuls out[j, (c,v)] = sum_t pk[t,j] * x16[t, (c,v)] (fp16, 1
cycle/row) accumulated over the 4 t-tiles into (128, 400) psum pieces
(<=512 f32: a matmul output cannot cross a 2KB psum bank). This removes
the Q7 ap_gather (Pool) entirely: no wrapped-index constants, no int32
packing, no 18.6us serial Pool gather in the tail. Gate scaling collapses
to a per-partition multiply (DVE/ACT alternating) at the psum->sbuf fp16
copy (gate2 = 2*sigmoid compensates the 0.5-scaled one-hots), since j is
the partition dim of the gather output.

Output is stored fp16 as (B, NEW_T, C, V) (contiguous 3200B runs) and the
host transposes back to (B, C, NEW_T, V) f32; output precision only sees
the fp16 payload rounding (~7e-4 end-to-end vs the 2e-2 gate). Scores and
top-k stay exact f32: fp16 x would perturb scores by ~2e-6 while the
reference's adjacent score gaps go down to 4.5e-9 (checked host-side), so
the f32 x load is irreducible.

DMA (the binding resource, 360GB/s aggregate in the cost model): 52.4MB x
load + 0.5MB W + 6.55MB fp16 store = 165.3us busy per core. Schedule: the
previous sample's chain/gather stages are emitted interleaved with the
next sample's four load sections so no in-order engine queue sees a burst
larger than the 6-chunk stage ring can buffer; stores for samples 0..B-2
are held in SBUF and sem-anchored on the first last-tile reduce so the
drain starts exactly at load end and covers the final sample's
chain+gather; the last sample's stores go out per-piece; PE warmup
matmuls anchored into the last tile's load stream keep the clock at full
p-state for the final chain+gather; the xn_c transpose copies both run
on ACT since the DVE reduce train is what paces the final chain.
Cost-model makespan 178.1us/core (c-major fp32-store baseline: 191.6us;
pure DMA floor ~167us).
"""

import math
import os
import sys

import numpy as np

for _p in ("/opt/trn_rl_repo", "/root/.axon_site/_ro/trn_rl_repo"):
    if os.path.isdir(_p) and _p not in sys.path:
        sys.path.insert(0, _p)

import concourse.mybir as mybir
import concourse.tile as tile
from concourse.masks import make_identity
from concourse.tile import add_dep_helper

# ---- problem constants (hardcoded per contract) ----
N, C, T, V = 32, 256, 512, 25
NEW_T = 128                      # ceil(T / K_POOL)
H = 8
HD = C // H
N_CORES = 8
B = N // N_CORES                 # samples per core
SCALE_S = 1.0 / (H * T * math.sqrt(HD))
ALPHA = SCALE_S / V

F32 = mybir.dt.float32
F16 = mybir.dt.float16
AX = mybir.AxisListType
OP = mybir.AluOpType
AF = mybir.ActivationFunctionType

P = 128                          # partitions
NCT = C // P                     # c-halves for the weight chain (2)
NTT = T // P                     # t tiles per sample (4)
CCH = 32                         # channels per load chunk
NCH = C // CCH                   # load chunks per t-tile (8)
SCW = 64                         # channels per output store piece
FCV = C * V                      # flattened (c, v) free size (6400)
PIECE = 400                      # gather psum piece free size (<=512 f32,
                                 # matmul output must stay in one psum bank)
NPC = FCV // PIECE               # psum pieces per sample (16)
STW = 4 * PIECE                  # store piece free size (1600)
NST = FCV // STW                 # stores per sample (4)


def emit_kernel(tc, nc, x_ap, w_ap, b_ap, o_ap, ctx, dbg=None):
    consts = ctx.enter_context(tc.tile_pool(name="consts", bufs=1))
    scratch = ctx.enter_context(tc.tile_pool(name="scratch", bufs=1))
    xstg = ctx.enter_context(tc.tile_pool(name="xstg", bufs=7))
    xt16p = ctx.enter_context(tc.tile_pool(name="xt16p", bufs=8))
    xntp = ctx.enter_context(tc.tile_pool(name="xntp", bufs=5))
    xncp = ctx.enter_context(tc.tile_pool(name="xncp", bufs=4))
    pkp = ctx.enter_context(tc.tile_pool(name="pkp", bufs=8))
    small = ctx.enter_context(tc.tile_pool(name="small", bufs=2))
    # 14 bufs suffice: the final sample's 4 stage tiles reuse sample-0
    # slots whose anchored stores drain ~10us before the last gather
    otp = ctx.enter_context(tc.tile_pool(name="otp", bufs=14))
    junkp = ctx.enter_context(tc.tile_pool(name="junkp", bufs=3))
    psum = ctx.enter_context(tc.tile_pool(name="psum", bufs=2, space="PSUM"))
    psums = ctx.enter_context(tc.tile_pool(name="psums", bufs=1,
                                           space="PSUM"))
    psumg = ctx.enter_context(tc.tile_pool(name="psumg", bufs=3,
                                           space="PSUM"))

    # ---------------- prologue: constants ----------------
    ident = consts.tile([P, P], F32)
    make_identity(nc, ident)

    ones_row = consts.tile([1, P], F32)
    nc.vector.memset(ones_row, 1.0)
    ones_col = consts.tile([P, 1], F32)
    nc.vector.memset(ones_col, 1.0)
    half_col = consts.tile([P, 1], F32)
    nc.vector.memset(half_col, 0.5)

    # iota_j row (1,128) fp32 and rank decode constant
    # P[t,j] = (rank == j) <=> (2j - 511 == signsum)
    iota_j = scratch.tile([1, P], F32, tag="iotaj")
    nc.gpsimd.iota(iota_j, pattern=[[1, P]], base=0, channel_multiplier=0,
                   allow_small_or_imprecise_dtypes=True)
    jb_ps = psum.tile([P, P], F32, tag="ps")
    nc.tensor.matmul(jb_ps, lhsT=ones_row, rhs=iota_j)
    iotaj2 = consts.tile([P, P], F32)
    nc.vector.tensor_scalar(iotaj2, jb_ps, 2.0, -511.0, op0=OP.mult,
                            op1=OP.add)

    # iotaT_k fp16 columns (128,1), values t = 128k + p (dbg idx extraction)
    iotaT16 = []
    if dbg is not None:
        for k in range(NTT):
            ff = consts.tile([P, 1], F16, tag=f"iotaT{k}")
            nc.gpsimd.iota(ff, pattern=[[0, 1]], base=P * k,
                           channel_multiplier=1,
                           allow_small_or_imprecise_dtypes=True)
            iotaT16.append(ff)

    # pre-load the ACT function tables (~1.3us each on first use) so the
    # first sample's chain doesn't pay them
    warm_in = consts.tile([P, 4], F32, tag="warm_in")
    nc.vector.memset(warm_in, 0.0)
    warm_out = consts.tile([P, 4], F32, tag="warm_out")
    for wf in (AF.Sign, AF.Abs, AF.Relu, AF.Sigmoid, AF.Identity):
        nc.scalar.activation(warm_out, warm_in, wf, bias=half_col[:, 0:1])

    # W rows load contiguously on the Pool SWDGE queue (2KB descriptors);
    # the q/k column compaction runs once the first tile streams
    wk_sb, TbkT, bqT, wqT = [], [], [], []
    wbox = []

    def emit_weights_dmas():
        for ct in range(NCT):
            wf = scratch.tile([P, 2 * C], F32, tag=f"wfull{ct}")
            nc.gpsimd.dma_start(out=wf, in_=w_ap[ct * P:(ct + 1) * P, :])
            wbox.append(wf)

    def emit_weights_prologue():
        b_view = b_ap.rearrange("(o h two i) -> o h two i", o=1, two=2, i=HD)
        for _ in range(NCT):
            wqT.append([None] * NCT)
        for ct in range(NCT):
            wf = wbox[ct]
            wv = wf.rearrange("p (h two i) -> p h two i", two=2, i=HD)
            wk = consts.tile([P, C], F32, tag=f"wk{ct}")
            nc.scalar.copy(wk.rearrange("p (h i) -> p h i", i=HD),
                           wv[:, :, 1, :])
            wk_sb.append(wk)
            wq = scratch.tile([P, C], F32, tag="wq")
            nc.scalar.copy(wq.rearrange("p (h i) -> p h i", i=HD),
                           wv[:, :, 0, :])
            for k2 in range(NCT):
                ps = psum.tile([P, P], F32, tag="ps")
                nc.tensor.transpose(ps, wq[:, k2 * P:(k2 + 1) * P], ident)
                t_ = consts.tile([P, P], F32, tag=f"wqT{k2}{ct}")
                nc.vector.tensor_copy(t_, ps)
                wqT[k2][ct] = t_

        bstage = scratch.tile([1, C], F32, tag="wqst")
        nc.gpsimd.dma_start(out=bstage, in_=b_view[0:1, :, 1, :])
        for k2 in range(NCT):
            ps = psum.tile([P, 1], F32, tag="ps")
            nc.tensor.transpose(ps, bstage[0:1, k2 * P:(k2 + 1) * P],
                                ident[0:1, 0:1])
            t_ = consts.tile([P, 1], F32, tag=f"TbkT{k2}")
            nc.vector.tensor_scalar(t_, ps, float(T), None, op0=OP.mult)
            TbkT.append(t_)
        bstage2 = scratch.tile([1, C], F32, tag="wqst")
        nc.gpsimd.dma_start(out=bstage2, in_=b_view[0:1, :, 0, :])
        for k2 in range(NCT):
            ps2 = psum.tile([P, 1], F32, tag="ps")
            nc.tensor.transpose(ps2, bstage2[0:1, k2 * P:(k2 + 1) * P],
                                ident[0:1, 0:1])
            t2 = consts.tile([P, 1], F32, tag=f"bqT{k2}")
            nc.vector.tensor_copy(t2, ps2)
            bqT.append(t2)


    emit_weights_dmas()

    # ---------------- per-sample pipeline ----------------
    held = []        # deferred store stage tiles, [(n, dma_emitter)...]
    store_anchors = []   # late sample-3 reduce instructions

    def emit_chain_a(n, xn_c, xnt_t):
        # ---- xsum columns via PE ones-matmuls over the xnT tiles: runs
        # concurrently with the xn_c transpose copies instead of waiting
        # for them (chain latency feeds the kernel tail) ----
        xs_ps = psum.tile([P, NCT], F32, tag="xsp", bufs=1)
        for ct in range(NCT):
            for k in range(NTT):
                nc.tensor.matmul(xs_ps[:, ct:ct + 1],
                                 lhsT=xnt_t[k][:, ct * P:(ct + 1) * P],
                                 rhs=ones_col,
                                 start=(k == 0), stop=(k == NTT - 1))
        xs2 = small.tile([P, NCT], F32, tag="xsum")
        nc.scalar.copy(xs2, xs_ps)
        xsum_c = [xs2[:, ct:ct + 1] for ct in range(NCT)]

        # ---- ksum^T columns ----
        ksumT = []
        for k2 in range(NCT):
            ps = psum.tile([P, 1], F32, tag="ps")
            for ct in range(NCT):
                nc.tensor.matmul(
                    ps, lhsT=wk_sb[ct][:, k2 * P:(k2 + 1) * P],
                    rhs=xsum_c[ct], start=(ct == 0), stop=(ct == NCT - 1))
            kt = small.tile([P, 1], F32, tag="ksumT")
            nc.vector.tensor_scalar(kt, ps, 1.0 / V, TbkT[k2][:, 0:1],
                                    op0=OP.mult, op1=OP.add)
            ksumT.append(kt)

        # ---- u columns (Wq @ ksum), broadcast along free for the fused
        # raw+broadcast matmul ----
        u_c = []
        for m in range(NCT):
            ps = psum.tile([P, 1], F32, tag="ps")
            for k2 in range(NCT):
                nc.tensor.matmul(ps, lhsT=wqT[k2][m], rhs=ksumT[k2],
                                 start=(k2 == 0), stop=(k2 == NCT - 1))
            ubc = small.tile([P, P], F32, tag="ubc")
            nc.scalar.copy(ubc, ps[:, 0:1].to_broadcast([P, P]))
            u_c.append(ubc)

        # ---- beta = scale_s * (bq . ksum) ----
        c0_ps = psum.tile([1, 1], F32, tag="ps")
        for k2 in range(NCT):
            nc.tensor.matmul(c0_ps, lhsT=ksumT[k2], rhs=bqT[k2],
                             start=(k2 == 0), stop=(k2 == NCT - 1))
        beta = small.tile([1, 1], F32, tag="beta")
        nc.scalar.mul(beta, c0_ps, SCALE_S)

        # ---- raw scores, broadcast to all partitions in one matmul; the
        # psum bank is freed immediately via a DVE copy to SBUF so the rank
        # pass reads SBUF and PSUM stays within its 8 banks ----
        sb_ps = psums.tile([P, T], F32, tag="sbps")
        for ct in range(NCT):
            nc.tensor.matmul(sb_ps, lhsT=u_c[ct], rhs=xn_c[ct],
                             start=(ct == 0), stop=(ct == NCT - 1))
        raw_sb_t = scratch.tile([P, 2 * C], F32, tag="wfull0")
        raw_sb = raw_sb_t[0:1, 0:T]
        nc.scalar.copy(raw_sb, sb_ps[0:1, :])
        return sb_ps, raw_sb, beta

    def emit_chain_b1(n, state):
        # ---- rank stage 1: score columns, negate staging, tiles k0/k1.
        # Split from stage 2 so the DVE rank burst never exceeds what a
        # load section's reduce budget can absorb ----
        sb_ps, raw_sb, beta = state
        st4 = psum.tile([P, NTT], F32, tag="stp", bufs=1)
        st_cols = []
        for k in range(NTT):
            st_ps = st4[:, k:k + 1]
            nc.tensor.transpose(st_ps, raw_sb[0:1, k * P:(k + 1) * P],
                                ident[0:1, 0:1])
            st_cols.append(st_ps)
        # negated score columns for the ACT Sign biases, staged to SBUF on
        # DVE ahead of its own rank tiles so ACT starts without queueing
        nsTf = {}
        for k in (1, 3):
            nf = small.tile([P, 1], F32, tag=f"nsTf{k}")
            nc.vector.tensor_scalar(nf, st_cols[k], -1.0, None, op0=OP.mult)
            nsTf[k] = nf
        p_tiles = [self_rank(n, k, st_cols, nsTf, sb_ps) for k in (0, 1)]
        return (sb_ps, raw_sb, beta, st4, st_cols, nsTf, p_tiles)

    def self_rank(n, k, st_cols, nsTf, sb_ps):
        st_ps = st_cols[k]
        pk = pkp.tile([P, P], F16, tag="pk")
        if k % 2 == 1:
            # ACT path: signsum = 2*rank - 511 via Sign-with-accum
            # (no ties; self term contributes 0), one-hot via
            # Relu(0.5 - |signsum - (2j-511)|) -> {0, 0.5}
            rank2 = small.tile([P, 1], F32, tag="rank2")
            gt_j = junkp.tile([P, T], F32, tag="junk")
            nc.scalar.activation(gt_j, sb_ps, AF.Sign,
                                 bias=nsTf[k][:, 0:1],
                                 accum_out=rank2)
            ad = small.tile([P, P], F32, tag="ad")
            nc.scalar.activation(ad, iotaj2, AF.Abs,
                                 bias=rank2[:, 0:1], scale=-1.0)
            nc.scalar.activation(pk, ad, AF.Relu, bias=half_col[:, 0:1],
                                 scale=-1.0)
        else:
            # DVE path (concurrent with the ACT k-tiles; the real TRN2
            # Pool engine supports neither TensorScalarPtr nor free-
            # axis reduce, so rank stays on these two engines)
            gt_d = junkp.tile([P, T], F32, tag="junk")
            rank = small.tile([P, 1], F32, tag="rankd")
            nc.vector.tensor_scalar(gt_d, sb_ps, st_ps[:, 0:1], None,
                                    op0=OP.is_gt, op1=OP.add,
                                    accum_out=rank)
            rank2x = small.tile([P, 1], F32, tag="rank2x")
            nc.vector.tensor_scalar(rank2x, rank, 2.0, -511.0,
                                    op0=OP.mult, op1=OP.add)
            nc.vector.tensor_scalar(pk, iotaj2, rank2x[:, 0:1], 0.5,
                                    op0=OP.is_equal, op1=OP.mult)
        return pk

    def emit_chain_b2(n, state2):
        # ---- rank stage 2 (tiles k2/k3), then values -> gate2 column ----
        sb_ps, raw_sb, beta, st4, st_cols, nsTf, p_tiles = state2
        p_tiles = list(p_tiles)
        for k in (2, 3):
            p_tiles.append(self_rank(n, k, st_cols, nsTf, sb_ps))
        # nsT emitted after the pk ops so the ACT queue reaches pks sooner
        nsTs = []
        for k in range(NTT):
            nsT = small.tile([P, 1], F16, tag="nsT", bufs=4)
            nc.scalar.mul(nsT, st_cols[k], -1.0)
            nsTs.append(nsT)
        p_tiles = [(p_tiles[k], nsTs[k]) for k in range(NTT)]
        val_ps = psum.tile([1, P], F32, tag="ps")
        for k in range(NTT):
            nc.tensor.matmul(val_ps, lhsT=p_tiles[k][1], rhs=p_tiles[k][0],
                             start=(k == 0), stop=(k == NTT - 1))
        gate = small.tile([1, P], F32, tag="gate")
        nc.scalar.activation(gate, val_ps, AF.Sigmoid, scale=-2.0 * ALPHA,
                             bias=beta[0:1, 0:1])
        g_ps = psum.tile([P, 1], F32, tag="ps")
        nc.tensor.transpose(g_ps, gate, ident[0:1, 0:1])
        gate2 = small.tile([P, 1], F32, tag="gate2")
        nc.scalar.mul(gate2, g_ps, 2.0)

        if dbg is not None:
            nc.sync.dma_start(out=dbg["scores"][n:n + 1, :], in_=raw_sb)
            nc.sync.dma_start(out=dbg["beta"][n:n + 1, :],
                              in_=beta[0:1, 0:1])
            nc.sync.dma_start(out=dbg["gate"][n:n + 1, :], in_=gate)
            idx_f = scratch.tile([1, P], F32, tag="idxf")
            idxr_ps = psum.tile([1, P], F32, tag="ps")
            for k in range(NTT):
                nc.tensor.matmul(idxr_ps, lhsT=iotaT16[k],
                                 rhs=p_tiles[k][0],
                                 start=(k == 0), stop=(k == NTT - 1))
            nc.scalar.mul(idx_f, idxr_ps, 2.0)
            nc.sync.dma_start(out=dbg["idx"][n:n + 1, :], in_=idx_f)

        return p_tiles, gate2

    def emit_gather_group(n, xt16_t, p_tiles, gate2, s, act_only=False):
        # ---- gather: out[j, (c,v)] = sum_t pk[t,j] x16[t, (c,v)] ----
        # psum pieces of 400 (one bank); four pieces share one fp16 store
        # stage of 1600; gate2 multiply+convert copies alternate DVE/ACT.
        # Groups are emitted interleaved with the NEXT sample's load
        # sections so the copies never block the in-order DVE/ACT queues
        # ahead of the reduce/convert stream for more than ~1.5us.
        ot = otp.tile([P, STW], F16, tag="ot")
        for h in range(4):
            pc = 4 * s + h
            gp = psumg.tile([P, 512], F32, tag="gp")
            gpv = gp[:, 0:PIECE]
            # accumulate in pk-arrival order so the most matmuls possible
            # pre-run before the last one-hot lands
            korder = (0, 1, 2, 3)
            for ki, k in enumerate(korder):
                nc.tensor.matmul(
                    gpv, lhsT=p_tiles[k][0],
                    rhs=xt16_t[k].rearrange("p c v -> p (c v)")
                        [:, pc * PIECE:(pc + 1) * PIECE],
                    start=(ki == 0), stop=(ki == NTT - 1))
            ots = ot[:, h * PIECE:(h + 1) * PIECE]
            if pc % 2 == 0 and not act_only:
                nc.vector.tensor_scalar(ots, gpv, gate2[:, 0:1], None,
                                        op0=OP.mult)
            else:
                nc.scalar.mul(ots, gpv, gate2[:, 0:1])
            if n == B - 1:
                # the final sample's stores go out per-piece so the drain
                # refills as soon as each copy lands (800B runs, no
                # small-descriptor penalty)
                o_flat = o_ap[n].rearrange("p c v -> p (c v)")
                nc.sync.dma_start(out=o_flat[:, pc * PIECE:(pc + 1) * PIECE],
                                  in_=ots)

        if n < B - 1:
            def store(n=n, s=s, ot=ot):
                return nc.sync.dma_start(
                    out=o_ap[n, :, s * SCW:(s + 1) * SCW, :]
                        .rearrange("p c v -> p (c v)"),
                    in_=ot)
            held.append((n, store))

    # ---- stream loads. Sample n-1's chain/gather is emitted in stages
    # interleaved with sample n's four t-tile load sections (stage A after
    # k0, rank after k1, gather groups 0-1 after k2 and 2-3 after k3), so
    # no engine-queue burst exceeds what the stg ring can buffer ----
    prev = None      # rolling state for sample n-1's chain stages
    warm_src = []    # (pk16, xt16) of sample B-2, for PE warmup
    for n in range(B):
        xt16_t = []
        xnt_t = []
        xn_c = [xncp.tile([P, T], F32, tag="xnc", name=f"xnc{n}_{i}")
                for i in range(NCT)]
        for k in range(NTT):
            xt16 = xt16p.tile([P, C, V], F16, tag="xt16")
            xnt = xntp.tile([P, C], F32, tag="xnt")
            for ch in range(NCH):
                c0 = ch * CCH
                stg = xstg.tile([P, CCH, V], F32, tag="stg")
                nc.sync.dma_start(
                    out=stg,
                    in_=x_ap[n, k * P:(k + 1) * P, c0:c0 + CCH, :])
                red = nc.vector.tensor_reduce(
                    out=xnt[:, c0:c0 + CCH], in_=stg, axis=AX.X, op=OP.add)
                # fp16 payload convert: mostly ACT, 3/8 on the idle Pool;
                # the last sample's final two tiles go entirely to Pool so
                # the ACT queue is clear when its score chain lands (the
                # converts only feed the gather, which runs later)
                if (n == B - 1 and k >= NTT - 2) or ch % 3 == 2:
                    nc.gpsimd.tensor_copy(xt16[:, c0:c0 + CCH, :], stg)
                else:
                    nc.scalar.copy(xt16[:, c0:c0 + CCH, :], stg)
                if n == B - 1 and k == NTT - 1:
                    store_anchors.append(red)
            # transpose xnT (t, c) into the c-major xn_c halves
            for ct in range(NCT):
                tp = psum.tile([P, P], F32, tag="ps")
                nc.tensor.transpose(tp, xnt[:, ct * P:(ct + 1) * P], ident)
                # both on ACT (GPSIMD cannot read PSUM; DVE is the
                # saturated engine - its reduce train paces the tail)
                nc.scalar.copy(xn_c[ct][:, k * P:(k + 1) * P], tp)
            xt16_t.append(xt16)
            xnt_t.append(xnt)
            if n == 0 and k == 0:
                with tc.tile_wait_until(0.012):
                    emit_weights_prologue()

            if prev is not None:
                # chain stages spread over sections: A at k0, rank k0/k1
                # at k1, rank k2/k3 + gate at k2 with the first two gather
                # groups (ACT-side copies), last two groups at k3. For the
                # last sample, k3's copies also go ACT-only so its final
                # reduces ride right behind the loads on a clear DVE.
                pn, pxt, pxn, pxnt, pstate, ptiles = prev
                last = n == B - 1
                if k == 0:
                    pstate = emit_chain_a(pn, pxn, pxnt)
                elif k == 1:
                    pstate = emit_chain_b1(pn, pstate)
                    ptiles = emit_chain_b2(pn, pstate)
                    if last:
                        emit_gather_group(pn, pxt, ptiles[0], ptiles[1], 0)
                        emit_gather_group(pn, pxt, ptiles[0], ptiles[1], 1)
                        warm_src.append((ptiles[0][0][0], pxt[0]))
                elif k == 2:
                    emit_gather_group(pn, pxt, ptiles[0], ptiles[1],
                                      2 if last else 0, act_only=last)
                    emit_gather_group(pn, pxt, ptiles[0], ptiles[1],
                                      3 if last else 1, act_only=last)
                else:
                    if not last:
                        emit_gather_group(pn, pxt, ptiles[0], ptiles[1], 2)
                        emit_gather_group(pn, pxt, ptiles[0], ptiles[1], 3)
                prev = (pn, pxt, pxn, pxnt, pstate, ptiles)
        if n < B - 1:
            prev = (n, xt16_t, xn_c, xnt_t, None, None)
        else:
            # PE p-state warmup: junk matmuls re-reading sample B-2's pk /
            # payload (ring-8 slots, never recycled by sample B-1), paced
            # by dep-anchors on the final tile's reduces so the engine is
            # at full clock when the last chain+gather hits it
            if warm_src and len(store_anchors) >= NCH:
                wpk, wxt = warm_src[-1]
                wps = psumg.tile([P, 512], F32, tag="gp")
                wxf = wxt.rearrange("p c v -> p (c v)")
                NW = 10
                for i in range(NW):
                    mm = nc.tensor.matmul(
                        wps, lhsT=wpk, rhs=wxf[:, 0:512],
                        start=(i == 0), stop=(i == NW - 1))
                    add_dep_helper(mm.ins,
                                   store_anchors[min(3 + i // 2,
                                                     NCH - 1)].ins,
                                   sync=True,
                                   reason="PE warmup before final gather")
            state = emit_chain_a(n, xn_c, xnt_t)
            state = emit_chain_b1(n, state)
            p_tiles, gate2 = emit_chain_b2(n, state)
            for s in range(NST):
                emit_gather_group(n, xt16_t, p_tiles, gate2, s)

    # ---------------- drain: deferred stores ----------------
    # samples 0..B-2 stores dep-anchored on a late sample-3 reduce so they
    # drain right at load end, covering the final sample's chain+gather
    anchor = store_anchors[0] if store_anchors else None
    for (n, store) in held:
        with tc.tile_wait_until(0.150):
            dma = store()
        if anchor is not None:
            add_dep_helper(dma.ins, anchor.ins, sync=True,
                           reason="deferred store drains at load end")


def build(debug_outs=False):
    import concourse.bacc as bacc
    nc = bacc.Bacc("TRN2", target_bir_lowering=False, debug=False)
    x_d = nc.dram_tensor("x", (B, T, C, V), F32, kind="ExternalInput")
    w_d = nc.dram_tensor("W", (C, 2 * C), F32, kind="ExternalInput")
    b_d = nc.dram_tensor("b", (2 * C,), F32, kind="ExternalInput")
    o_d = nc.dram_tensor("out", (B, NEW_T, C, V), F16, kind="ExternalOutput")
    dbg = None
    if debug_outs:
        dbg = {
            "scores": nc.dram_tensor("dbg_scores", (B, T), F32,
                                     kind="ExternalOutput").ap(),
            "gate": nc.dram_tensor("dbg_gate", (B, P), F32,
                                   kind="ExternalOutput").ap(),
            "idx": nc.dram_tensor("dbg_idx", (B, P), F32,
                                  kind="ExternalOutput").ap(),
            "beta": nc.dram_tensor("dbg_beta", (B, 1), F32,
                                   kind="ExternalOutput").ap(),
        }
    from contextlib import ExitStack
    with tile.TileContext(nc) as tc:
        with ExitStack() as ctx:
            emit_kernel(tc, nc, x_d.ap(), w_d.ap(), b_d.ap(), o_d.ap(), ctx,
                        dbg=dbg)
    nc.compile()
    return nc


_NC_CACHE = {}


def get_nc(debug_outs=False):
    if debug_outs not in _NC_CACHE:
        _NC_CACHE[debug_outs] = build(debug_outs)
    return _NC_CACHE[debug_outs]


def make_in_maps(x, W, b):
    x = np.ascontiguousarray(
        np.asarray(x, dtype=np.float32).transpose(0, 2, 1, 3))
    W = np.ascontiguousarray(W, dtype=np.float32)
    b = np.ascontiguousarray(b, dtype=np.float32)
    return [{"x": x[c * B:(c + 1) * B], "W": W, "b": b}
            for c in range(N_CORES)]


def run(in_maps, trace=False, debug_outs=False):
    from concourse.bass_utils import run_bass_kernel_spmd
    return run_bass_kernel_spmd(get_nc(debug_outs), in_maps,
                                core_ids=list(range(N_CORES)), trace=trace)


def kernel(**inputs):
    res = run(make_in_maps(inputs["x"], inputs["W"], inputs["b"]))
    out = np.concatenate([res.results[c]["out"] for c in range(N_CORES)],
                         axis=0)
    # (N, NEW_T, C, V) fp16 -> (N, C, NEW_T, V) f32
    return np.ascontiguousarray(out.transpose(0, 2, 1, 3)).astype(np.float32)
